# revision 31
# baseline (speedup 1.0000x reference)
"""Trainium2 Bass kernel for nn_Loss_9749575762182.

Computes two scalar losses over (8192, 2048) fp32 tensors:
  wmse = mean((weight[:,None] * (target - input))**2)
  wcl  = mean(|(st*ln(tp+eps) + (1-st)*ln(1-tp+eps)) * obrT|)

Strategy: data-parallel over the row axis across 8 NeuronCores
(1024 rows each), eight [128, 2048] tiles per core streamed through
SBUF. The tiny per-partition partial sums land back in DRAM and the
host finishes the reduction in float64.

Performance history (profiled core-0 exec): 181us baseline ->
121-122us typical (occasional ~141us when 8-core HBM contention is
unlucky). The 41.94MB/core input stream runs the 16 SDMA engines at
~100% line rate (~400+ GB/s, ~102us) and is the roofline; everything
else hides behind it except ~6us init/preamble and ~8us tail.

Key algebraic restructure: obrT >= 0, st in [0,1], and both logs are
<= 0, so
  |bce*ob| = -bce*ob = |u*l1| + |v*l2|,  u = st*ob, v = (1-st)*ob
which splits the old serial chain (Ln2->sub->mul->add->mul->Abs,
18.7us/tile ping-pong) into two INDEPENDENT depth-1 chains after the
Lns. u and v' = (st-1)*ob depend only on DMA'd inputs (v' via one
scalar_tensor_tensor op), and products are accumulated directly on
DVE via scalar_tensor_tensor(accum_out=...), removing the ACT Abs
pass entirely:
  ACT: l1 = Ln(tp + eps); l2 = Ln(-tp + (1+eps)); Square(w*diff)+accum
  DVE: diff = tgt - in; u = st*ob; v' = (st-1)*ob [STT];
       a = l1*u  +accum (<= 0);  b = l2*v' +accum (>= 0)
  wcl_sum = -sum(a) + sum(b)   (signs resolved on the host)

Scheduling lessons that each bought 10-25us (DO NOT regress these):
  - LAG the a/b accumulation by one tile: DVE runs a(t-1)/b(t-1)
    between diff(t) and u(t), so no DVE op ever waits on an ACT op of
    its own tile. Without the lag, any ACT-order choice couples the
    engines into a ~15.5us/tile serial cadence (vs 11.3us busy).
  - ACT program order per tile must be Ln1, Ln2, Square. Square
    before the Lns puts it between q's landing and the tail chain and
    serializes the last tiles (+20us).
  - DMA batch order g,x,s,o,q matches DVE consumption order; q last
    keeps the post-stream drain to Ln -> a-stt -> b-stt. The last
    tile's q is loaded in nsp=2 column halves with half-width
    Ln/accum ops to shorten the drain (nsp=4 measured WORSE).
  - ALL big loads on the single qSP HWDGE ring (nc.sync). The ring
    runs all 16 SDMA engines at line rate by itself. Splitting s,o
    onto SWDGE (nc.gpsimd) made engines interleave queues at packet
    granularity and cost ~28% extra engine-time per byte (165us).
    SP dispatch is ring-credit paced (~9 DMAs in flight, ~22us
    dispatch-to-land latency); buffer gates (bufs=3) never bind.

Hard-won environment notes (axon-tunneled trn2, this toolchain):
  - Build on bacc.Bacc() and call nc.finalize() before run_bass_via_pjrt;
    raw bass.Bass() BIR fails walrus ("Reg has not been allocated"), and
    without Bacc's generate_event_semaphores pass any instruction with
    >1 semaphore wait dies in codegen ("Too many sync wait commands").
  - tensor_tensor_reduce compiles + simulates fine but faults on real HW
    via the PJRT path; scalar_tensor_tensor with accum_out is the DVE
    accumulation that does work (ACT activation accum_out also works).
  - The Tile list-scheduler reorders instructions within an engine by
    estimated ready time; python issue order is a hint, not law.
  - 8-core HBM contention adds 0-25% engine-busy inflation run to run;
    re-bench 3x before believing any <10us delta.
"""

import os
import sys

if "/opt/trn_rl_repo" not in sys.path:
    sys.path.insert(0, "/opt/trn_rl_repo")

import numpy as np

N, D = 8192, 2048
NCORES = 8
ROWS = N // NCORES  # rows per core
P = 128             # SBUF partitions
EPS = 1e-10

_CACHE = {}


def build(rows=ROWS, d=D, bufs=3):
    import concourse.bacc as bacc
    import concourse.tile as tile
    from concourse import mybir

    f32 = mybir.dt.float32
    ALU = mybir.AluOpType
    ACTF = mybir.ActivationFunctionType
    nt = rows // P
    # the last tile's q/Ln/accum chain is split into two column halves,
    # so the cl accumulator has one extra column per term
    ca = nt + 1  # accum columns per cl term

    nc = bacc.Bacc()
    inp = nc.dram_tensor("input", [rows, d], f32, kind="ExternalInput")
    tgt = nc.dram_tensor("target", [rows, d], f32, kind="ExternalInput")
    wgt = nc.dram_tensor("weight", [rows], f32, kind="ExternalInput")
    st = nc.dram_tensor("sub_target", [rows, d], f32, kind="ExternalInput")
    tp = nc.dram_tensor("target_pre", [rows, d], f32, kind="ExternalInput")
    ob = nc.dram_tensor("sub_obrT", [rows, d], f32, kind="ExternalInput")
    # mse partials (ACT-written); cl partials (DVE-written): a-term in
    # cols [0, ca), b-term in cols [ca, 2*ca)
    out_mse = nc.dram_tensor("mse_partials", [P, nt], f32, kind="ExternalOutput")
    out_cl = nc.dram_tensor("cl_partials", [P, 2 * ca], f32, kind="ExternalOutput")

    inp_t = inp.rearrange("(t p) d -> t p d", p=P)
    tgt_t = tgt.rearrange("(t p) d -> t p d", p=P)
    st_t = st.rearrange("(t p) d -> t p d", p=P)
    tp_t = tp.rearrange("(t p) d -> t p d", p=P)
    ob_t = ob.rearrange("(t p) d -> t p d", p=P)
    wgt_t = wgt.rearrange("(t p) -> p t", p=P)

    with tile.TileContext(nc) as tc:
        with (
            tc.tile_pool(name="singles", bufs=1) as singles,
            tc.tile_pool(name="in_p", bufs=bufs) as in_p,
            tc.tile_pool(name="tgt_p", bufs=bufs) as tgt_p,
            tc.tile_pool(name="st_p", bufs=bufs) as st_p,
            tc.tile_pool(name="tp_p", bufs=bufs) as tp_p,
            tc.tile_pool(name="ob_p", bufs=bufs) as ob_p,
            tc.tile_pool(name="l1_p", bufs=2) as l1_p,
            tc.tile_pool(name="l2_p", bufs=2) as l2_p,
            tc.tile_pool(name="u_p", bufs=2) as u_p,
            tc.tile_pool(name="v_p", bufs=2) as v_p,
        ):
            # w_cols via SWDGE: keeps the tiny strided load (and its many
            # small descriptors) off the head of the qSP HWDGE FIFO that
            # streams the 40 big tile loads.
            w_cols = singles.tile([P, nt], f32)
            nc.gpsimd.dma_start(out=w_cols, in_=wgt_t)
            mse_cols = singles.tile([P, nt], f32)
            cl_cols = singles.tile([P, 2 * ca], f32)
            eps_b = singles.tile([P, 1], f32)
            nc.vector.memset(eps_b, EPS)
            one_eps_b = singles.tile([P, 1], f32)
            nc.vector.memset(one_eps_b, 1.0 + EPS)

            # Each instruction should depend on at most ONE foreign
            # semaphore that is not yet known-satisfied; tiny "touch" ops
            # consume extra waits so the real consumers inherit them via
            # engine program order / already-observed clocks.
            touch_d = singles.tile([P, 1], f32)
            atouch_d = singles.tile([P, 1], f32)
            nc.scalar.activation(
                out=atouch_d, in_=w_cols[:, 0:1], func=ACTF.Copy
            )  # waits w_cols DMA on ACT
            nc.scalar.activation(
                out=atouch_d, in_=eps_b, func=ACTF.Copy
            )  # waits DVE memsets on ACT
            nc.scalar.activation(out=atouch_d, in_=one_eps_b, func=ACTF.Copy)

            # Pipeline shape (steady state, DMA-paced at ~12.7us/tile):
            #   DMA batch order g,x,s,o,q matches DVE op order, so DVE is
            #   never input-starved. The a/b accumulation ops LAG ONE TILE:
            #   a(t-1)/b(t-1) consume Ln outputs finished a full tile ago,
            #   so no DVE op ever waits on an ACT op of its own tile -- the
            #   cross-engine chains that serialized earlier versions
            #   (~15.5us/tile) are gone. DVE (~11.3us/tile) rides just
            #   behind the free-running 414 GB/s DMA stream.
            # The LAST tile's q is loaded in two halves and its Ln/accum
            # chain runs half-width, shrinking the post-stream drain from
            # ~6.5us to ~4.5us.
            prev = None
            for t in range(nt - 1):
                # All big loads on the single qSP HWDGE ring: it alone runs
                # the 16 SDMA engines at ~100% line rate (~400 GB/s).
                # Splitting across SWDGE (or any second queue) makes the
                # engines interleave queues at packet granularity and COSTS
                # ~28% extra engine time per byte (measured).
                g = tgt_p.tile([P, d], f32)
                nc.sync.dma_start(out=g, in_=tgt_t[t])
                x = in_p.tile([P, d], f32)
                nc.sync.dma_start(out=x, in_=inp_t[t])
                s = st_p.tile([P, d], f32)
                nc.sync.dma_start(out=s, in_=st_t[t])
                o = ob_p.tile([P, d], f32)
                nc.sync.dma_start(out=o, in_=ob_t[t])
                q = tp_p.tile([P, d], f32)
                nc.sync.dma_start(out=q, in_=tp_t[t])

                # ---- wmse: diff on DVE, then Square(w*diff)+accum on ACT
                nc.vector.tensor_copy(touch_d, g[:, 0:1])  # consume g-DMA wait
                nc.vector.tensor_sub(g, g, x)  # g <- diff = target - input

                # ---- lagged accumulation of the previous tile's products
                if prev is not None:
                    pl1, pl2, pu, pv, pt = prev
                    nc.vector.scalar_tensor_tensor(
                        out=pl1,  # sink, in place
                        in0=pl1,
                        scalar=0.0,
                        in1=pu,
                        op0=ALU.bypass,
                        op1=ALU.mult,
                        accum_out=cl_cols[:, pt : pt + 1],
                    )
                    nc.vector.scalar_tensor_tensor(
                        out=pl2,  # sink, in place
                        in0=pl2,
                        scalar=0.0,
                        in1=pv,
                        op0=ALU.bypass,
                        op1=ALU.mult,
                        accum_out=cl_cols[:, ca + pt : ca + pt + 1],
                    )

                # ---- wcl inputs: u = st*ob; v' = (st-1)*ob (both pre-Ln)
                nc.vector.tensor_copy(touch_d, s[:, 0:1])  # consume s-DMA wait
                u = u_p.tile([P, d], f32)
                nc.vector.tensor_mul(u, s, o)
                v = v_p.tile([P, d], f32)
                nc.vector.scalar_tensor_tensor(
                    out=v,  # v <- v' = (st - 1) * ob
                    in0=s,
                    scalar=1.0,
                    in1=o,
                    op0=ALU.subtract,
                    op1=ALU.mult,
                )

                # ---- logs (ACT); bias/scale fold the affine into the LUT
                l1 = l1_p.tile([P, d], f32)
                l2 = l2_p.tile([P, d], f32)
                nc.scalar.activation(
                    out=l1, in_=q, func=ACTF.Ln, bias=eps_b, scale=1.0
                )
                nc.scalar.activation(
                    out=l2, in_=q, func=ACTF.Ln, bias=one_eps_b, scale=-1.0
                )
                # Square AFTER the Lns in ACT program order: it feeds only
                # the mse store, so it must not sit between q landing and
                # the Ln->a/b tail chain.
                nc.scalar.activation(
                    out=x,  # sink; x is dead after the sub
                    in_=g,
                    func=ACTF.Square,
                    bias=0.0,
                    scale=w_cols[:, t : t + 1],
                    accum_out=mse_cols[:, t : t + 1],
                )
                prev = (l1, l2, u, v, t)

            # ---- LAST tile: batch landing order == consumption order, so
            # everything except Ln-h2 -> a-h2 -> b-h2 (and Square, off the
            # cl path) completes BEFORE the final byte lands. Tail after
            # the last byte is ~3.5us instead of ~8.5us.
            t = nt - 1
            d2 = d // 2
            s = st_p.tile([P, d], f32)
            nc.sync.dma_start(out=s, in_=st_t[t])
            o = ob_p.tile([P, d], f32)
            nc.sync.dma_start(out=o, in_=ob_t[t])
            q = tp_p.tile([P, d], f32)
            nc.sync.dma_start(out=q[:, 0:d2], in_=tp_t[t][:, 0:d2])
            g = tgt_p.tile([P, d], f32)
            nc.sync.dma_start(out=g, in_=tgt_t[t])
            x = in_p.tile([P, d], f32)
            nc.sync.dma_start(out=x, in_=inp_t[t])
            nc.sync.dma_start(out=q[:, d2:d], in_=tp_t[t][:, d2:d])

            # lagged pair for tile nt-2 (deps long satisfied)
            pl1, pl2, pu, pv, pt = prev
            nc.vector.scalar_tensor_tensor(
                out=pl1, in0=pl1, scalar=0.0, in1=pu,
                op0=ALU.bypass, op1=ALU.mult,
                accum_out=cl_cols[:, pt : pt + 1],
            )
            nc.vector.scalar_tensor_tensor(
                out=pl2, in0=pl2, scalar=0.0, in1=pv,
                op0=ALU.bypass, op1=ALU.mult,
                accum_out=cl_cols[:, ca + pt : ca + pt + 1],
            )

            # u, v' as soon as s,o land
            nc.vector.tensor_copy(touch_d, s[:, 0:1])
            u = u_p.tile([P, d], f32)
            nc.vector.tensor_mul(u, s, o)
            v = v_p.tile([P, d], f32)
            nc.vector.scalar_tensor_tensor(
                out=v, in0=s, scalar=1.0, in1=o,
                op0=ALU.subtract, op1=ALU.mult,
            )

            # h1 logs + accums (q-h1 lands mid-batch)
            l1 = l1_p.tile([P, d], f32)
            l2 = l2_p.tile([P, d], f32)
            nc.scalar.activation(
                out=l1[:, 0:d2], in_=q[:, 0:d2], func=ACTF.Ln,
                bias=eps_b, scale=1.0,
            )
            nc.scalar.activation(
                out=l2[:, 0:d2], in_=q[:, 0:d2], func=ACTF.Ln,
                bias=one_eps_b, scale=-1.0,
            )
            nc.vector.scalar_tensor_tensor(
                out=l1[:, 0:d2], in0=l1[:, 0:d2], scalar=0.0, in1=u[:, 0:d2],
                op0=ALU.bypass, op1=ALU.mult,
                accum_out=cl_cols[:, t : t + 1],
            )
            nc.vector.scalar_tensor_tensor(
                out=l2[:, 0:d2], in0=l2[:, 0:d2], scalar=0.0, in1=v[:, 0:d2],
                op0=ALU.bypass, op1=ALU.mult,
                accum_out=cl_cols[:, ca + t : ca + t + 1],
            )

            # diff while q-h2 is still in flight
            nc.vector.tensor_copy(touch_d, g[:, 0:1])
            nc.vector.tensor_sub(g, g, x)

            # h2 logs + accums: the only post-stream work on the cl path
            nc.scalar.activation(
                out=l1[:, d2:d], in_=q[:, d2:d], func=ACTF.Ln,
                bias=eps_b, scale=1.0,
            )
            nc.scalar.activation(
                out=l2[:, d2:d], in_=q[:, d2:d], func=ACTF.Ln,
                bias=one_eps_b, scale=-1.0,
            )
            nc.vector.scalar_tensor_tensor(
                out=l1[:, d2:d], in0=l1[:, d2:d], scalar=0.0, in1=u[:, d2:d],
                op0=ALU.bypass, op1=ALU.mult,
                accum_out=cl_cols[:, t + 1 : t + 2],
            )
            nc.vector.scalar_tensor_tensor(
                out=l2[:, d2:d], in0=l2[:, d2:d], scalar=0.0, in1=v[:, d2:d],
                op0=ALU.bypass, op1=ALU.mult,
                accum_out=cl_cols[:, ca + t + 1 : ca + t + 2],
            )

            # Square runs on ACT in parallel with the h2 accums (mse only)
            nc.scalar.activation(
                out=x, in_=g, func=ACTF.Square, bias=0.0,
                scale=w_cols[:, t : t + 1],
                accum_out=mse_cols[:, t : t + 1],
            )

            # cl store first: its gate (b-h2) releases before Square's
            nc.sync.dma_start(out=out_cl[:, :], in_=cl_cols)
            nc.sync.dma_start(out=out_mse[:, :], in_=mse_cols)
    return nc


def _get_nc():
    bufs = int(os.environ.get("BASS_BUFS", "3"))
    if bufs not in _CACHE:
        nc = build(bufs=bufs)
        nc.finalize()  # runs Bacc's passes (event-sem wait splitting, regalloc)
        _CACHE[bufs] = nc
    return _CACHE[bufs]


def _install_profile_hook():
    """Register the NTFF profile hook that this container's stripped antenv
    lacks: a ctypes bridge into libaxon_pjrt.so (same ABI trn_boot.py uses).
    Only needed for trace=True runs."""
    if "antenv.axon_hooks" in sys.modules:
        return
    import contextlib
    import ctypes
    import types

    so_path = "/opt/axon/libaxon_pjrt.so"
    lib = ctypes.CDLL(so_path)
    if not hasattr(lib, "axon_start_nrt_profile"):
        return
    lib.axon_start_nrt_profile.argtypes = [
        ctypes.POINTER(ctypes.c_int64),
        ctypes.c_size_t,
    ]
    lib.axon_start_nrt_profile.restype = ctypes.c_int64
    lib.axon_stop_nrt_profile.argtypes = [ctypes.c_char_p]
    lib.axon_stop_nrt_profile.restype = ctypes.c_int64

    @contextlib.contextmanager
    def _hook(output_dir, device_ids):
        import jax

        jax.devices()
        if device_ids:
            ids = (ctypes.c_int64 * len(device_ids))(*device_ids)
            rc = lib.axon_start_nrt_profile(ids, len(device_ids))
        else:
            rc = lib.axon_start_nrt_profile(None, 0)
        if rc != 0:
            raise RuntimeError(f"axon_start_nrt_profile rc={rc}")
        try:
            yield
        finally:
            n = lib.axon_stop_nrt_profile(str(output_dir).encode())
            print(f"profile: {n} file(s) written to {output_dir}")

    mod = types.ModuleType("antenv.axon_hooks")
    mod.get_axon_ntff_profile_hook = lambda: _hook
    sys.modules["antenv.axon_hooks"] = mod


def kernel(**inputs):
    from concourse.bass_utils import run_bass_kernel_spmd

    nc = _get_nc()
    names = ["input", "target", "weight", "sub_target", "target_pre", "sub_obrT"]
    arrs = {k: np.ascontiguousarray(np.asarray(inputs[k], dtype=np.float32)) for k in names}
    in_maps = []
    for c in range(NCORES):
        sl = slice(c * ROWS, (c + 1) * ROWS)
        in_maps.append({k: np.ascontiguousarray(v[sl]) for k, v in arrs.items()})

    trace = os.environ.get("BASS_KERNEL_PROFILE", "0") == "1"
    if trace:
        _install_profile_hook()
    res = run_bass_kernel_spmd(nc, in_maps, list(range(NCORES)), trace=trace)

    ca = ROWS // P + 1  # matches build()
    mse_sum = 0.0
    cla_sum = 0.0
    clb_sum = 0.0
    for r in res.results:
        mse_sum += np.asarray(r["mse_partials"], dtype=np.float64).sum()
        cl = np.asarray(r["cl_partials"], dtype=np.float64)
        cla_sum += cl[:, :ca].sum()
        clb_sum += cl[:, ca:].sum()
    tot = float(N) * float(D)
    if trace and res.exec_time_ns is not None:
        print(f"HW exec time: {res.exec_time_ns} ns")
    return (
        np.asarray(np.float32(mse_sum / tot)),
        np.asarray(np.float32((clb_sum - cla_sum) / tot)),
    )


# revision 34
# speedup vs baseline: 1.1686x; 1.1686x over previous
"""Trainium2 Bass kernel for nn_Loss_9749575762182.

Computes two scalar losses over (8192, 2048) fp32 tensors:
  wmse = mean((weight[:,None] * (target - input))**2)
  wcl  = mean(|(st*ln(tp+eps) + (1-st)*ln(1-tp+eps)) * obrT|)

Strategy: data-parallel over the row axis across 8 NeuronCores
(1024 rows each), eight [128, 2048] tiles per core streamed through
SBUF. The tiny per-partition partial sums land back in DRAM and the
host finishes the reduction in float64.

Performance history (profiled core-0 exec): 181us baseline ->
121-122us typical (occasional ~141us when 8-core HBM contention is
unlucky). The 41.94MB/core input stream runs the 16 SDMA engines at
~100% line rate (~400+ GB/s, ~102us) and is the roofline; everything
else hides behind it except ~6us init/preamble and ~8us tail.

Key algebraic restructure: obrT >= 0, st in [0,1], and both logs are
<= 0, so
  |bce*ob| = -bce*ob = |u*l1| + |v*l2|,  u = st*ob, v = (1-st)*ob
which splits the old serial chain (Ln2->sub->mul->add->mul->Abs,
18.7us/tile ping-pong) into two INDEPENDENT depth-1 chains after the
Lns. u and v' = (st-1)*ob depend only on DMA'd inputs (v' via one
scalar_tensor_tensor op), and products are accumulated directly on
DVE via scalar_tensor_tensor(accum_out=...), removing the ACT Abs
pass entirely:
  ACT: l1 = Ln(tp + eps); l2 = Ln(-tp + (1+eps)); Square(w*diff)+accum
  DVE: diff = tgt - in; u = st*ob; v' = (st-1)*ob [STT];
       a = l1*u  +accum (<= 0);  b = l2*v' +accum (>= 0)
  wcl_sum = -sum(a) + sum(b)   (signs resolved on the host)

Scheduling lessons that each bought 10-25us (DO NOT regress these):
  - LAG the a/b accumulation by one tile: DVE runs a(t-1)/b(t-1)
    between diff(t) and u(t), so no DVE op ever waits on an ACT op of
    its own tile. Without the lag, any ACT-order choice couples the
    engines into a ~15.5us/tile serial cadence (vs 11.3us busy).
  - ACT program order per tile must be Ln1, Ln2, Square. Square
    before the Lns puts it between q's landing and the tail chain and
    serializes the last tiles (+20us).
  - DMA batch order g,x,s,o,q matches DVE consumption order; q last
    keeps the post-stream drain to Ln -> a-stt -> b-stt. The last
    tile's q is loaded in nsp=2 column halves with half-width
    Ln/accum ops to shorten the drain (nsp=4 measured WORSE).
  - ALL big loads on the single qSP HWDGE ring (nc.sync). The ring
    runs all 16 SDMA engines at line rate by itself. Splitting s,o
    onto SWDGE (nc.gpsimd) made engines interleave queues at packet
    granularity and cost ~28% extra engine-time per byte (165us).
    SP dispatch is ring-credit paced (~9 DMAs in flight, ~22us
    dispatch-to-land latency); buffer gates (bufs=3) never bind.

Hard-won environment notes (axon-tunneled trn2, this toolchain):
  - Build on bacc.Bacc() and call nc.finalize() before run_bass_via_pjrt;
    raw bass.Bass() BIR fails walrus ("Reg has not been allocated"), and
    without Bacc's generate_event_semaphores pass any instruction with
    >1 semaphore wait dies in codegen ("Too many sync wait commands").
  - tensor_tensor_reduce compiles + simulates fine but faults on real HW
    via the PJRT path; scalar_tensor_tensor with accum_out is the DVE
    accumulation that does work (ACT activation accum_out also works).
  - The Tile list-scheduler reorders instructions within an engine by
    estimated ready time; python issue order is a hint, not law.
  - 8-core HBM contention adds 0-25% engine-busy inflation run to run;
    re-bench 3x before believing any <10us delta.
"""

import os
import sys

if "/opt/trn_rl_repo" not in sys.path:
    sys.path.insert(0, "/opt/trn_rl_repo")

import numpy as np

N, D = 8192, 2048
NCORES = 8
ROWS = N // NCORES  # rows per core
P = 128             # SBUF partitions
EPS = 1e-10

_CACHE = {}


def build(rows=ROWS, d=D, bufs=3):
    import concourse.bacc as bacc
    import concourse.tile as tile
    from concourse import mybir

    f32 = mybir.dt.float32
    ALU = mybir.AluOpType
    ACTF = mybir.ActivationFunctionType
    nt = rows // P
    # the last tile's q/Ln/accum chain is split into two column halves,
    # so the cl accumulator has one extra column per term
    ca = nt + 1  # accum columns per cl term

    nc = bacc.Bacc()
    inp = nc.dram_tensor("input", [rows, d], f32, kind="ExternalInput")
    tgt = nc.dram_tensor("target", [rows, d], f32, kind="ExternalInput")
    wgt = nc.dram_tensor("weight", [rows], f32, kind="ExternalInput")
    st = nc.dram_tensor("sub_target", [rows, d], f32, kind="ExternalInput")
    tp = nc.dram_tensor("target_pre", [rows, d], f32, kind="ExternalInput")
    ob = nc.dram_tensor("sub_obrT", [rows, d], f32, kind="ExternalInput")
    # mse partials (ACT-written); cl partials (DVE-written): a-term in
    # cols [0, ca), b-term in cols [ca, 2*ca)
    out_mse = nc.dram_tensor("mse_partials", [P, nt], f32, kind="ExternalOutput")
    out_cl = nc.dram_tensor("cl_partials", [P, 2 * ca], f32, kind="ExternalOutput")

    inp_t = inp.rearrange("(t p) d -> t p d", p=P)
    tgt_t = tgt.rearrange("(t p) d -> t p d", p=P)
    st_t = st.rearrange("(t p) d -> t p d", p=P)
    tp_t = tp.rearrange("(t p) d -> t p d", p=P)
    ob_t = ob.rearrange("(t p) d -> t p d", p=P)
    wgt_t = wgt.rearrange("(t p) -> p t", p=P)

    with tile.TileContext(nc) as tc:
        with (
            tc.tile_pool(name="singles", bufs=1) as singles,
            tc.tile_pool(name="in_p", bufs=bufs) as in_p,
            tc.tile_pool(name="tgt_p", bufs=bufs) as tgt_p,
            tc.tile_pool(name="st_p", bufs=bufs) as st_p,
            tc.tile_pool(name="tp_p", bufs=bufs) as tp_p,
            tc.tile_pool(name="ob_p", bufs=bufs) as ob_p,
            tc.tile_pool(name="l1_p", bufs=2) as l1_p,
            tc.tile_pool(name="l2_p", bufs=2) as l2_p,
            tc.tile_pool(name="u_p", bufs=2) as u_p,
            tc.tile_pool(name="v_p", bufs=2) as v_p,
        ):
            # w_cols via SWDGE: keeps the tiny strided load (and its many
            # small descriptors) off the head of the qSP HWDGE FIFO that
            # streams the 40 big tile loads.
            w_cols = singles.tile([P, nt], f32)
            nc.gpsimd.dma_start(out=w_cols, in_=wgt_t)
            mse_cols = singles.tile([P, nt], f32)
            cl_cols = singles.tile([P, 2 * ca], f32)
            eps_b = singles.tile([P, 1], f32)
            nc.vector.memset(eps_b, EPS)
            one_eps_b = singles.tile([P, 1], f32)
            nc.vector.memset(one_eps_b, 1.0 + EPS)

            # Each instruction should depend on at most ONE foreign
            # semaphore that is not yet known-satisfied; tiny "touch" ops
            # consume extra waits so the real consumers inherit them via
            # engine program order / already-observed clocks.
            touch_d = singles.tile([P, 1], f32)
            atouch_d = singles.tile([P, 1], f32)
            nc.scalar.activation(
                out=atouch_d, in_=w_cols[:, 0:1], func=ACTF.Copy
            )  # waits w_cols DMA on ACT
            nc.scalar.activation(
                out=atouch_d, in_=eps_b, func=ACTF.Copy
            )  # waits DVE memsets on ACT
            nc.scalar.activation(out=atouch_d, in_=one_eps_b, func=ACTF.Copy)

            # Pipeline shape (steady state, DMA-paced at ~12.7us/tile):
            #   DMA batch order g,x,s,o,q matches DVE op order, so DVE is
            #   never input-starved. The a/b accumulation ops LAG ONE TILE:
            #   a(t-1)/b(t-1) consume Ln outputs finished a full tile ago,
            #   so no DVE op ever waits on an ACT op of its own tile -- the
            #   cross-engine chains that serialized earlier versions
            #   (~15.5us/tile) are gone. DVE (~11.3us/tile) rides just
            #   behind the free-running 414 GB/s DMA stream.
            # The LAST tile's q is loaded in two halves and its Ln/accum
            # chain runs half-width, shrinking the post-stream drain from
            # ~6.5us to ~4.5us.
            prev = None
            for t in range(nt - 2):
                # All big loads on the single qSP HWDGE ring: it alone runs
                # the 16 SDMA engines at ~100% line rate (~400 GB/s).
                # Splitting across SWDGE (or any second queue) makes the
                # engines interleave queues at packet granularity and COSTS
                # ~28% extra engine time per byte (measured).
                g = tgt_p.tile([P, d], f32)
                nc.sync.dma_start(out=g, in_=tgt_t[t])
                x = in_p.tile([P, d], f32)
                nc.sync.dma_start(out=x, in_=inp_t[t])
                s = st_p.tile([P, d], f32)
                nc.sync.dma_start(out=s, in_=st_t[t])
                o = ob_p.tile([P, d], f32)
                nc.sync.dma_start(out=o, in_=ob_t[t])
                q = tp_p.tile([P, d], f32)
                nc.sync.dma_start(out=q, in_=tp_t[t])

                # ---- wmse: diff on DVE, then Square(w*diff)+accum on ACT
                nc.vector.tensor_copy(touch_d, g[:, 0:1])  # consume g-DMA wait
                nc.vector.tensor_sub(g, g, x)  # g <- diff = target - input

                # ---- lagged accumulation of the previous tile's products
                if prev is not None:
                    pl1, pl2, pu, pv, pt = prev
                    nc.vector.scalar_tensor_tensor(
                        out=pl1,  # sink, in place
                        in0=pl1,
                        scalar=0.0,
                        in1=pu,
                        op0=ALU.bypass,
                        op1=ALU.mult,
                        accum_out=cl_cols[:, pt : pt + 1],
                    )
                    nc.vector.scalar_tensor_tensor(
                        out=pl2,  # sink, in place
                        in0=pl2,
                        scalar=0.0,
                        in1=pv,
                        op0=ALU.bypass,
                        op1=ALU.mult,
                        accum_out=cl_cols[:, ca + pt : ca + pt + 1],
                    )

                # ---- wcl inputs: u = st*ob; v' = (st-1)*ob (both pre-Ln)
                nc.vector.tensor_copy(touch_d, s[:, 0:1])  # consume s-DMA wait
                u = u_p.tile([P, d], f32)
                nc.vector.tensor_mul(u, s, o)
                v = v_p.tile([P, d], f32)
                nc.vector.scalar_tensor_tensor(
                    out=v,  # v <- v' = (st - 1) * ob
                    in0=s,
                    scalar=1.0,
                    in1=o,
                    op0=ALU.subtract,
                    op1=ALU.mult,
                )

                # ---- logs (ACT); bias/scale fold the affine into the LUT
                l1 = l1_p.tile([P, d], f32)
                l2 = l2_p.tile([P, d], f32)
                nc.scalar.activation(
                    out=l1, in_=q, func=ACTF.Ln, bias=eps_b, scale=1.0
                )
                nc.scalar.activation(
                    out=l2, in_=q, func=ACTF.Ln, bias=one_eps_b, scale=-1.0
                )
                # Square AFTER the Lns in ACT program order: it feeds only
                # the mse store, so it must not sit between q landing and
                # the Ln->a/b tail chain.
                nc.scalar.activation(
                    out=x,  # sink; x is dead after the sub
                    in_=g,
                    func=ACTF.Square,
                    bias=0.0,
                    scale=w_cols[:, t : t + 1],
                    accum_out=mse_cols[:, t : t + 1],
                )
                prev = (l1, l2, u, v, t)

            # ---- Tile nt-2: q loads EARLY so its Lns (and therefore its
            # a/b accums, run UN-lagged here) complete well before the
            # stream ends -- this removes 4.4us of DVE work from the
            # final-tile window, which is DVE-throughput-bound.
            t = nt - 2
            s = st_p.tile([P, d], f32)
            nc.sync.dma_start(out=s, in_=st_t[t])
            o = ob_p.tile([P, d], f32)
            nc.sync.dma_start(out=o, in_=ob_t[t])
            q = tp_p.tile([P, d], f32)
            nc.sync.dma_start(out=q, in_=tp_t[t])
            g = tgt_p.tile([P, d], f32)
            nc.sync.dma_start(out=g, in_=tgt_t[t])
            x = in_p.tile([P, d], f32)
            nc.sync.dma_start(out=x, in_=inp_t[t])

            # lagged pair for tile nt-3
            pl1, pl2, pu, pv, pt = prev
            nc.vector.scalar_tensor_tensor(
                out=pl1, in0=pl1, scalar=0.0, in1=pu,
                op0=ALU.bypass, op1=ALU.mult,
                accum_out=cl_cols[:, pt : pt + 1],
            )
            nc.vector.scalar_tensor_tensor(
                out=pl2, in0=pl2, scalar=0.0, in1=pv,
                op0=ALU.bypass, op1=ALU.mult,
                accum_out=cl_cols[:, ca + pt : ca + pt + 1],
            )

            nc.vector.tensor_copy(touch_d, s[:, 0:1])
            u = u_p.tile([P, d], f32)
            nc.vector.tensor_mul(u, s, o)
            v = v_p.tile([P, d], f32)
            nc.vector.scalar_tensor_tensor(
                out=v, in0=s, scalar=1.0, in1=o,
                op0=ALU.subtract, op1=ALU.mult,
            )
            l1 = l1_p.tile([P, d], f32)
            l2 = l2_p.tile([P, d], f32)
            nc.scalar.activation(out=l1, in_=q, func=ACTF.Ln, bias=eps_b, scale=1.0)
            nc.scalar.activation(
                out=l2, in_=q, func=ACTF.Ln, bias=one_eps_b, scale=-1.0
            )
            # un-lagged accums for THIS tile (Lns are ready early)
            nc.vector.scalar_tensor_tensor(
                out=l1, in0=l1, scalar=0.0, in1=u,
                op0=ALU.bypass, op1=ALU.mult,
                accum_out=cl_cols[:, t : t + 1],
            )
            nc.vector.scalar_tensor_tensor(
                out=l2, in0=l2, scalar=0.0, in1=v,
                op0=ALU.bypass, op1=ALU.mult,
                accum_out=cl_cols[:, ca + t : ca + t + 1],
            )
            nc.vector.tensor_copy(touch_d, g[:, 0:1])
            nc.vector.tensor_sub(g, g, x)
            nc.scalar.activation(
                out=x, in_=g, func=ACTF.Square, bias=0.0,
                scale=w_cols[:, t : t + 1],
                accum_out=mse_cols[:, t : t + 1],
            )

            # ---- LAST tile: batch landing order == consumption order, so
            # everything except Ln-h2 -> a-h2 -> b-h2 (and Square, off the
            # cl path) completes BEFORE the final byte lands.
            t = nt - 1
            d2 = d // 2
            s = st_p.tile([P, d], f32)
            nc.sync.dma_start(out=s, in_=st_t[t])
            o = ob_p.tile([P, d], f32)
            nc.sync.dma_start(out=o, in_=ob_t[t])
            q = tp_p.tile([P, d], f32)
            nc.sync.dma_start(out=q[:, 0:d2], in_=tp_t[t][:, 0:d2])
            g = tgt_p.tile([P, d], f32)
            nc.sync.dma_start(out=g, in_=tgt_t[t])
            x = in_p.tile([P, d], f32)
            nc.sync.dma_start(out=x, in_=inp_t[t])
            nc.sync.dma_start(out=q[:, d2:d], in_=tp_t[t][:, d2:d])

            # u, v' as soon as s,o land
            nc.vector.tensor_copy(touch_d, s[:, 0:1])
            u = u_p.tile([P, d], f32)
            nc.vector.tensor_mul(u, s, o)
            v = v_p.tile([P, d], f32)
            nc.vector.scalar_tensor_tensor(
                out=v, in0=s, scalar=1.0, in1=o,
                op0=ALU.subtract, op1=ALU.mult,
            )

            # h1 logs + accums (q-h1 lands mid-batch)
            l1 = l1_p.tile([P, d], f32)
            l2 = l2_p.tile([P, d], f32)
            nc.scalar.activation(
                out=l1[:, 0:d2], in_=q[:, 0:d2], func=ACTF.Ln,
                bias=eps_b, scale=1.0,
            )
            nc.scalar.activation(
                out=l2[:, 0:d2], in_=q[:, 0:d2], func=ACTF.Ln,
                bias=one_eps_b, scale=-1.0,
            )
            nc.vector.scalar_tensor_tensor(
                out=l1[:, 0:d2], in0=l1[:, 0:d2], scalar=0.0, in1=u[:, 0:d2],
                op0=ALU.bypass, op1=ALU.mult,
                accum_out=cl_cols[:, t : t + 1],
            )
            nc.vector.scalar_tensor_tensor(
                out=l2[:, 0:d2], in0=l2[:, 0:d2], scalar=0.0, in1=v[:, 0:d2],
                op0=ALU.bypass, op1=ALU.mult,
                accum_out=cl_cols[:, ca + t : ca + t + 1],
            )

            # diff while q-h2 is still in flight
            nc.vector.tensor_copy(touch_d, g[:, 0:1])
            nc.vector.tensor_sub(g, g, x)

            # h2 logs + accums: the only post-stream work on the cl path
            nc.scalar.activation(
                out=l1[:, d2:d], in_=q[:, d2:d], func=ACTF.Ln,
                bias=eps_b, scale=1.0,
            )
            nc.scalar.activation(
                out=l2[:, d2:d], in_=q[:, d2:d], func=ACTF.Ln,
                bias=one_eps_b, scale=-1.0,
            )
            nc.vector.scalar_tensor_tensor(
                out=l1[:, d2:d], in0=l1[:, d2:d], scalar=0.0, in1=u[:, d2:d],
                op0=ALU.bypass, op1=ALU.mult,
                accum_out=cl_cols[:, t + 1 : t + 2],
            )
            nc.vector.scalar_tensor_tensor(
                out=l2[:, d2:d], in0=l2[:, d2:d], scalar=0.0, in1=v[:, d2:d],
                op0=ALU.bypass, op1=ALU.mult,
                accum_out=cl_cols[:, ca + t + 1 : ca + t + 2],
            )

            # Square runs on ACT in parallel with the h2 accums (mse only)
            nc.scalar.activation(
                out=x, in_=g, func=ACTF.Square, bias=0.0,
                scale=w_cols[:, t : t + 1],
                accum_out=mse_cols[:, t : t + 1],
            )

            # cl store first: its gate (b-h2) releases before Square's
            nc.sync.dma_start(out=out_cl[:, :], in_=cl_cols)
            nc.sync.dma_start(out=out_mse[:, :], in_=mse_cols)
    return nc


def _get_nc():
    bufs = int(os.environ.get("BASS_BUFS", "3"))
    if bufs not in _CACHE:
        nc = build(bufs=bufs)
        nc.finalize()  # runs Bacc's passes (event-sem wait splitting, regalloc)
        _CACHE[bufs] = nc
    return _CACHE[bufs]


def _install_profile_hook():
    """Register the NTFF profile hook that this container's stripped antenv
    lacks: a ctypes bridge into libaxon_pjrt.so (same ABI trn_boot.py uses).
    Only needed for trace=True runs."""
    if "antenv.axon_hooks" in sys.modules:
        return
    import contextlib
    import ctypes
    import types

    so_path = "/opt/axon/libaxon_pjrt.so"
    lib = ctypes.CDLL(so_path)
    if not hasattr(lib, "axon_start_nrt_profile"):
        return
    lib.axon_start_nrt_profile.argtypes = [
        ctypes.POINTER(ctypes.c_int64),
        ctypes.c_size_t,
    ]
    lib.axon_start_nrt_profile.restype = ctypes.c_int64
    lib.axon_stop_nrt_profile.argtypes = [ctypes.c_char_p]
    lib.axon_stop_nrt_profile.restype = ctypes.c_int64

    @contextlib.contextmanager
    def _hook(output_dir, device_ids):
        import jax

        jax.devices()
        if device_ids:
            ids = (ctypes.c_int64 * len(device_ids))(*device_ids)
            rc = lib.axon_start_nrt_profile(ids, len(device_ids))
        else:
            rc = lib.axon_start_nrt_profile(None, 0)
        if rc != 0:
            raise RuntimeError(f"axon_start_nrt_profile rc={rc}")
        try:
            yield
        finally:
            n = lib.axon_stop_nrt_profile(str(output_dir).encode())
            print(f"profile: {n} file(s) written to {output_dir}")

    mod = types.ModuleType("antenv.axon_hooks")
    mod.get_axon_ntff_profile_hook = lambda: _hook
    sys.modules["antenv.axon_hooks"] = mod


def kernel(**inputs):
    from concourse.bass_utils import run_bass_kernel_spmd

    nc = _get_nc()
    names = ["input", "target", "weight", "sub_target", "target_pre", "sub_obrT"]
    arrs = {k: np.ascontiguousarray(np.asarray(inputs[k], dtype=np.float32)) for k in names}
    in_maps = []
    for c in range(NCORES):
        sl = slice(c * ROWS, (c + 1) * ROWS)
        in_maps.append({k: np.ascontiguousarray(v[sl]) for k, v in arrs.items()})

    trace = os.environ.get("BASS_KERNEL_PROFILE", "0") == "1"
    if trace:
        _install_profile_hook()
    res = run_bass_kernel_spmd(nc, in_maps, list(range(NCORES)), trace=trace)

    ca = ROWS // P + 1  # matches build()
    mse_sum = 0.0
    cla_sum = 0.0
    clb_sum = 0.0
    for r in res.results:
        mse_sum += np.asarray(r["mse_partials"], dtype=np.float64).sum()
        cl = np.asarray(r["cl_partials"], dtype=np.float64)
        cla_sum += cl[:, :ca].sum()
        clb_sum += cl[:, ca:].sum()
    tot = float(N) * float(D)
    if trace and res.exec_time_ns is not None:
        print(f"HW exec time: {res.exec_time_ns} ns")
    return (
        np.asarray(np.float32(mse_sum / tot)),
        np.asarray(np.float32((clb_sum - cla_sum) / tot)),
    )
